# revision 40
# baseline (speedup 1.0000x reference)
"""Trainium2 Bass kernel for per-sample conv self-attention.

Reference computation (per batch sample b, N = H*W = 4096, C = 64, C8 = 8):
    q = x @ wq + bq            [N, 8]
    k = x @ wk + bk            [N, 8]
    v = x @ wv + bv            [N, 64]
    attn = softmax(q @ k^T)    [N, N]   (softmax over keys, no scaling)
    out  = attn @ v * gamma + x

Sharding: data-parallel over batch — 8 samples onto 8 NeuronCores, one
sample per core.  Inside a core the attention matrix is processed
flash-style (never materialized in HBM).

Schedule (v2 — the steady state is co-limited by the exp() engines,
ScalarE (N+352)/1.2 ns and VectorE (N+120)/0.96 ns per [128,N] PSUM tile,
and by the PE):

  * Preamble interleaves with the first block: x quarter-DMAs land, each
    quarter is PE-transposed straight from fp32 (no bf16 pre-cast) into
    xT, q/k/v projections follow, and compute pairs for block 0 are
    emitted as soon as the chunks they need are staged.  Warm matmuls run
    only while the first DMA is in flight, so the HAM clock-gate opens
    during the preamble instead of 50us into the run.
  * S^T blocks are computed in 4-up row-tiled bursts: the K=8 matmuls for
    4 key chunks go to PE row strips 0/32/64/96 concurrently (kT/qT are
    replicated at all four partition offsets by the projection matmuls
    themselves).  One burst fills two [128,1024] fp32 PSUM tiles.
  * exp() is split across engines per pair: one tile on ScalarE (table
    exp) and one on VectorE (one-pass Schraudolph exp: i16 = S*128/ln2 +
    bias via tensor_scalar, bitcast to bf16; +-3% weight error that
    cancels in the softmax normalization).  Block-end pairs give ScalarE
    both tiles (it is the faster engine; 72/56 split overall).  No
    row-max subtraction is needed: |S| <= ~30 so exp stays in range.
  * out^T [65, 512] accumulates in PSUM as v'.T @ E^T where v' has a ones
    column appended - row 64 of out^T is then the softmax denominator
    (v itself is pre-scaled by gamma, the ones column is not).  The PV
    matmuls for a pair are emitted as one contiguous LDW/MM chain so the
    PE's background weight buffer hides the interior weight loads; they
    lag the S bursts by 2 pairs.
  * PSUM budget (8 banks): 3 st tiles x 2 banks + 2 oacc banks.  The
    preamble borrows st tiles for its transposes/projections, and the
    finale transposes write into the *previous* block's freed oacc bank,
    so no extra PSUM pools exist.
  * Finale per 128-query chunk: PE-transpose out^T back to [128, 65]
    fp32, then VectorE computes y = out * (1/denom) + x in one
    reciprocal + one scalar_tensor_tensor and DMAs out.
"""

from contextlib import ExitStack

import numpy as np

import concourse.bass as bass
import concourse.mybir as mybir
import concourse.tile as tile
from concourse import bacc
from concourse.bass_utils import run_bass_kernel_spmd
from concourse.masks import make_identity

F32 = mybir.dt.float32
BF16 = mybir.dt.bfloat16
I16 = mybir.dt.int16
AF = mybir.ActivationFunctionType
ALU = mybir.AluOpType

B, H, W, C = 8, 64, 64, 64
N = H * W          # 4096 pixels (queries == keys)
C8 = C // 8        # 8  qk head dim
NB = 8             # query blocks
NBLK = N // NB     # 512 queries per block
MCH = N // 128     # 32 m-chunks of 128 keys
NPAIR = NB * 8     # 64 pairs; pair = (block nb, 4 chunks)

# Schraudolph exp in bf16 domain: i16 = round(S * 2^7/ln2 + (127*2^7 - Cc)),
# bitcast int16 -> bf16 gives exp(S) with <=3.3% relative error (linear
# mantissa interpolation).  Cc centers the error; +0.5 pre-biases for the
# truncating float->int convert.  Valid for |S| <= 88; here |S| <= ~31 so
# i16 stays in [10531, 21981] - no overflow, no negatives, no denormals.
EXP_A16 = 128.0 / float(np.log(2.0))
EXP_B16 = 16256.0 - 366393.0 / 65536.0 + 0.5


def _body(nc, tc, io):
    x_d, wq_d, bq_d, wk_d, bk_d, wv_d, bv_d, gamma_d, y_d = io

    ctx = ExitStack()
    singles = ctx.enter_context(tc.tile_pool(name="singles", bufs=1))
    ident32 = singles.tile([128, 128], F32)      # identity for fp32 PE transposes
    x_sb = singles.tile([128, MCH * C], F32)     # resident x, chunk j at cols 64j
    xT = singles.tile([C + 1, N], BF16)          # x^T with ones row 64
    qT_rep = singles.tile([128, N], BF16)        # q^T replicated at parts 0/32/64/96
    kT_rep = singles.tile([128, N], BF16)        # k^T replicated at parts 0/32/64/96
    v_all = singles.tile([128, MCH * (C + 1)], BF16)  # v'_j at cols 65j, ones col 64
    gamma_sb = singles.tile([128, 1], F32)
    wq_st = singles.tile([C + 1, C8], F32)
    wk_st = singles.tile([C + 1, C8], F32)
    wv_st = singles.tile([C + 1, C], F32)
    wqp = singles.tile([C + 1, 128], BF16)       # wq' replicated into cols 0/32/64/96
    wkp = singles.tile([C + 1, 128], BF16)
    wvp = singles.tile([C + 1, C], BF16)
    warm_w = singles.tile([128, 128], BF16)

    # ---------------- input DMAs (issued before ANY other work) ----------
    # GpSimd does no DMAs: its queue is [xT-ones (3.5us, single Q7 core),
    # identity memset, affine_select] - all done by ~4.5us, before the
    # first transpose needs the identity.
    nc.gpsimd.memset(xT[C : C + 1, :], 1.0)
    for r, qeng in enumerate((nc.sync, nc.scalar, nc.scalar, nc.sync)):
        qeng.dma_start(
            out=x_sb[:, 512 * r : 512 * (r + 1)].rearrange(
                "p (c f) -> p c f", f=C
            ),
            in_=x_d[1024 * r : 1024 * (r + 1), :].rearrange(
                "(c p) f -> p c f", p=128
            ),
        )
    nc.scalar.dma_start(out=gamma_sb[:], in_=gamma_d.to_broadcast((128, 1)))
    nc.sync.dma_start(out=wq_st[0:C, :], in_=wq_d)
    nc.sync.dma_start(out=wq_st[C : C + 1, :], in_=bq_d)
    nc.sync.dma_start(out=wk_st[0:C, :], in_=wk_d)
    nc.sync.dma_start(out=wk_st[C : C + 1, :], in_=bk_d)
    nc.sync.dma_start(out=wv_st[0:C, :], in_=wv_d)
    nc.sync.dma_start(out=wv_st[C : C + 1, :], in_=bv_d)

    make_identity(nc, ident32)
    nc.vector.memset(warm_w[:], 0.0)

    # weight staging -> bf16, replicated into PE column groups (zero padded)
    nc.vector.memset(wqp[:], 0.0)
    nc.vector.memset(wkp[:], 0.0)
    for i in range(4):
        nc.vector.tensor_copy(out=wqp[:, 32 * i : 32 * i + C8], in_=wq_st[:])
        nc.vector.tensor_copy(out=wkp[:, 32 * i : 32 * i + C8], in_=wk_st[:])
    # fold gamma into the v projection: out^T accumulates gamma*v while
    # the denominator row (ones column of v') stays unscaled, so the
    # finale is just  y = out*(1/denom) + x.
    nc.vector.tensor_scalar(
        out=wvp[:], in0=wv_st[:], scalar1=gamma_sb[0 : C + 1, :], scalar2=None,
        op0=ALU.mult,
    )

    # ones columns of v' (strided FD=32 - cheap on VectorE)
    nc.vector.memset(
        v_all[:].rearrange("p (c f) -> p c f", f=C + 1)[:, :, C : C + 1], 1.0
    )

    # ---------------- PSUM pools (8 banks exactly) ----------------
    st_pool = ctx.enter_context(tc.tile_pool(name="st", bufs=3, space="PSUM"))
    out_pool = ctx.enter_context(tc.tile_pool(name="oacc", bufs=2, space="PSUM"))
    et_pool = ctx.enter_context(tc.tile_pool(name="et", bufs=10))
    ob_pool = ctx.enter_context(tc.tile_pool(name="ob", bufs=2))
    fin_pool = ctx.enter_context(tc.tile_pool(name="fin", bufs=2))

    # PE HAM warmup: real matmuls while the first x quarter is in flight
    # (transposes don't count as PE activity for the clock gate).  The warm
    # tile borrows the oacc pool's first rotation slot; its banks are
    # reclaimed by block 1's accumulator.
    warm_ps = out_pool.tile([128, NBLK], F32, tag="oacc", name="warm")

    def _warm(n):
        for _ in range(n):
            nc.tensor.matmul(
                warm_ps[:, 0:128], warm_w[:], warm_w[:], start=True, stop=True
            )

    _warm(16)

    # ---------------- staging of one x quarter ----------------
    def stage_quarter(r):
        # 8 fp32 transposes into one borrowed st tile, then one split
        # copy-cast to xT bf16 (ScalarE low half, VectorE high half).
        pt = st_pool.tile([128, 1024], F32, tag="st", name=f"pt{r}")
        for h in range(8):
            j = 8 * r + h
            nc.tensor.transpose(
                pt[0:C, 128 * h : 128 * (h + 1)],
                x_sb[:, C * j : C * (j + 1)], ident32[:],
            )
            if h % 2 == 1:
                # transposes don't count as PE activity for the HAM clock
                # gate; pulse a real matmul between pairs
                _warm(1)
        nc.scalar.copy(
            out=xT[0:C, 1024 * r : 1024 * r + 512], in_=pt[0:C, 0:512]
        )
        nc.vector.tensor_copy(
            out=xT[0:C, 1024 * r + 512 : 1024 * (r + 1)], in_=pt[0:C, 512:1024]
        )
        # q/k projections: output partitions carry the 4 replicas directly
        pq = st_pool.tile([128, 1024], F32, tag="st", name=f"pq{r}")
        for b2 in range(2):
            bsl = slice(1024 * r + 512 * b2, 1024 * r + 512 * (b2 + 1))
            nc.tensor.matmul(
                pq[:, 512 * b2 : 512 * (b2 + 1)], wqp[:], xT[:, bsl],
                start=True, stop=True,
            )
        csl = slice(1024 * r, 1024 * (r + 1))
        nc.scalar.copy(out=qT_rep[:, csl], in_=pq[:])
        pk = st_pool.tile([128, 1024], F32, tag="st", name=f"pk{r}")
        for b2 in range(2):
            bsl = slice(1024 * r + 512 * b2, 1024 * r + 512 * (b2 + 1))
            nc.tensor.matmul(
                pk[:, 512 * b2 : 512 * (b2 + 1)], wkp[:], xT[:, bsl],
                start=True, stop=True,
            )
        nc.vector.tensor_copy(out=kT_rep[:, csl], in_=pk[:])
        # v projections: 8 chunks into one borrowed st tile, split copy
        pv = st_pool.tile([128, 1024], F32, tag="st", name=f"pv{r}")
        for h in range(8):
            j = 8 * r + h
            nc.tensor.matmul(
                pv[:, C * h : C * (h + 1)],
                xT[:, 128 * j : 128 * (j + 1)], wvp[:],
                start=True, stop=True,
            )
        nc.scalar.copy(
            out=v_all[:].rearrange("p (c f) -> p c f", f=C + 1)[
                :, 8 * r : 8 * r + 4, 0:C
            ],
            in_=pv[:, 0 : 4 * C].rearrange("p (c f) -> p c f", f=C),
        )
        nc.vector.tensor_copy(
            out=v_all[:].rearrange("p (c f) -> p c f", f=C + 1)[
                :, 8 * r + 4 : 8 * r + 8, 0:C
            ],
            in_=pv[:, 4 * C : 8 * C].rearrange("p (c f) -> p c f", f=C),
        )

    # ---------------- main loop pieces ----------------
    oaccs = {}          # nb -> oacc tile (also yt scratch for nb-1 finales)
    pending_finales = []  # (due_pair, nb, ob_tile, k4)

    # exp engine per (pair, half).  Tile A (allocated first) is the one the
    # next pair's burst reuses, so it always goes to ScalarE (the faster
    # engine, and not loaded with ob/finale work) for the earliest WAR
    # release.  Block-end pairs give ScalarE both tiles (72/56 split),
    # except the last one (the drain wants the engines two-wide).
    def exp_engines(p):
        nb, gp = divmod(p, 8)
        if gp == 7 and p < NPAIR - 8:
            return ("S", "S")
        return ("S", "V")

    def emit_s_exp(p):
        nb, gp = divmod(p, 8)
        nsl = slice(nb * NBLK, (nb + 1) * NBLK)
        engines = exp_engines(p)
        sts = []
        for h in range(2):
            st = st_pool.tile([128, 1024], F32, tag="st")
            for i2 in range(2):
                i = 2 * h + i2
                j = 4 * gp + i
                nc.tensor.matmul(
                    st[:, i2 * NBLK : (i2 + 1) * NBLK],
                    kT_rep[32 * i : 32 * i + C8, 128 * j : 128 * (j + 1)],
                    qT_rep[32 * i : 32 * i + C8, nsl],
                    start=True, stop=True,
                    tile_position=(32 * i, 0),
                )
            sts.append(st)
        ets = []
        for h in range(2):
            et = et_pool.tile([128, 1024], BF16, tag="et")
            if engines[h] == "S":
                nc.scalar.activation(out=et[:], in_=sts[h][:], func=AF.Exp)
            else:
                nc.vector.tensor_scalar(
                    out=et[:].bitcast(I16), in0=sts[h][:],
                    scalar1=EXP_A16, scalar2=EXP_B16,
                    op0=ALU.mult, op1=ALU.add,
                )
            ets.append(et)
        return ets

    def emit_pv(p, ets):
        nb, gp = divmod(p, 8)
        if gp == 0:
            oaccs[nb] = out_pool.tile(
                [128, NBLK], F32, tag="oacc", name=f"oacc{nb}"
            )
        oacc = oaccs[nb]
        for h in range(2):
            for i2 in range(2):
                j = 4 * gp + 2 * h + i2
                nc.tensor.matmul(
                    oacc[0 : C + 1, :],
                    v_all[:, (C + 1) * j : (C + 1) * (j + 1)],
                    ets[h][:, i2 * NBLK : (i2 + 1) * NBLK],
                    start=(j == 0), stop=(j == MCH - 1),
                    skip_group_check=True,
                )
        if gp == 7:
            ob = ob_pool.tile([C + 1, NBLK], F32, tag="ob")
            # ScalarE: VectorE carries the finale recip/stt work
            nc.scalar.copy(out=ob[:], in_=oacc[0 : C + 1, :])
            return nb, ob
        return None

    def emit_finale(nb, ob, k4, yt=None):
        # transpose back into the freed oacc bank of block nb (its ob copy
        # is done by the time finales run).  yt overrides the target (the
        # drain uses freed st tiles so no two finales share a PSUM bank).
        if yt is None:
            yt = oaccs[nb][:, 65 * k4 : 65 * k4 + 65]
        nc.tensor.transpose(
            yt, ob[:, 128 * k4 : 128 * (k4 + 1)], ident32[0 : C + 1, 0 : C + 1]
        )
        rc = fin_pool.tile([128, 1], F32, tag="rc")
        nc.vector.reciprocal(rc[:], yt[:, C : C + 1])
        yo = fin_pool.tile([128, C], F32, tag="yo")
        ck = nb * 4 + k4
        nc.vector.scalar_tensor_tensor(
            out=yo[:], in0=yt[:, 0:C], scalar=rc[:],
            in1=x_sb[:, C * ck : C * (ck + 1)],
            op0=ALU.mult, op1=ALU.add,
        )
        nc.sync.dma_start(out=y_d[128 * ck : 128 * (ck + 1), :], in_=yo[:])

    # ---------------- emission schedule ----------------
    # pair slot p: [due finales] [PV(p-lag)] [S burst p + exps]
    # quarter r stages just before the pairs that first need it
    # (pair p needs kT/v chunks 4p..4p+3 -> quarter p//2 for block 0).
    prevs = []  # queue of (p, ets)

    def flush_pv(upto_excl):
        while prevs and prevs[0][0] < upto_excl:
            pp, pets = prevs.pop(0)
            res = emit_pv(pp, pets)
            if res is not None:
                fnb, fob = res
                base = pp + 5
                for t in range(4):
                    pending_finales.append((base + t, fnb, fob, t))

    for p in range(NPAIR):
        if p < 8 and p % 2 == 0:
            stage_quarter(p // 2)
            _warm(4)  # keep the HAM clock gate open through the ramp
        lag = 3 if p < NPAIR - 6 else (2 if p < NPAIR - 3 else 1)
        flush_pv(p - lag + 1)
        ets = emit_s_exp(p)
        prevs.append((p, ets))
        if p < 11:
            _warm(2)  # warm tile's bank is reclaimed by block 1's oacc
        # finales go after the burst: their transpose waits on the previous
        # finale's stt (same PSUM bank), which by now has drained
        while pending_finales and pending_finales[0][0] <= p:
            _, fnb, fob, fk4 = pending_finales.pop(0)
            emit_finale(fnb, fob, fk4)

    flush_pv(NPAIR)
    # drain: the st pool is idle now - park each remaining finale's yt in
    # its own PSUM bank so the transposes never wait on each other's reads
    drain_tiles = [
        st_pool.tile([128, 1024], F32, tag="st", name=f"dr{i}") for i in range(2)
    ]
    for di, (_, fnb, fob, fk4) in enumerate(pending_finales):
        if fnb == NB - 1:
            yt = drain_tiles[di // 2][:, 512 * (di % 2) : 512 * (di % 2) + 65]
            emit_finale(fnb, fob, fk4, yt=yt)
        else:
            emit_finale(fnb, fob, fk4)

    ctx.close()


def build_program():
    nc = bacc.Bacc("TRN2", target_bir_lowering=False, debug=False, num_devices=8)
    x_d = nc.dram_tensor("x", [N, C], F32, kind="ExternalInput").ap()
    wq_d = nc.dram_tensor("wq", [C, C8], F32, kind="ExternalInput").ap()
    bq_d = nc.dram_tensor("bq", [1, C8], F32, kind="ExternalInput").ap()
    wk_d = nc.dram_tensor("wk", [C, C8], F32, kind="ExternalInput").ap()
    bk_d = nc.dram_tensor("bk", [1, C8], F32, kind="ExternalInput").ap()
    wv_d = nc.dram_tensor("wv", [C, C], F32, kind="ExternalInput").ap()
    bv_d = nc.dram_tensor("bv", [1, C], F32, kind="ExternalInput").ap()
    gamma_d = nc.dram_tensor("gamma", [1, 1], F32, kind="ExternalInput").ap()
    y_d = nc.dram_tensor("y", [N, C], F32, kind="ExternalOutput").ap()

    io = (x_d, wq_d, bq_d, wk_d, bk_d, wv_d, bv_d, gamma_d, y_d)
    with tile.TileContext(nc) as tc:
        _body(nc, tc, io)
    nc.compile()
    return nc


_CACHE = {}


def _get_program():
    if "nc" not in _CACHE:
        _CACHE["nc"] = build_program()
    return _CACHE["nc"]


def make_in_maps(inputs):
    x = np.ascontiguousarray(np.asarray(inputs["x"], dtype=np.float32))
    wq = np.ascontiguousarray(np.asarray(inputs["wq"], dtype=np.float32))
    bq = np.asarray(inputs["bq"], dtype=np.float32).reshape(1, C8)
    wk = np.ascontiguousarray(np.asarray(inputs["wk"], dtype=np.float32))
    bk = np.asarray(inputs["bk"], dtype=np.float32).reshape(1, C8)
    wv = np.ascontiguousarray(np.asarray(inputs["wv"], dtype=np.float32))
    bv = np.asarray(inputs["bv"], dtype=np.float32).reshape(1, C)
    gamma = np.asarray(inputs["gamma"], dtype=np.float32).reshape(1, 1)
    in_maps = []
    for b in range(B):
        in_maps.append(
            {
                "x": np.ascontiguousarray(x[b].reshape(N, C)),
                "wq": wq, "bq": bq, "wk": wk, "bk": bk,
                "wv": wv, "bv": bv, "gamma": gamma,
            }
        )
    return in_maps


def run(inputs, **kwargs):
    nc = _get_program()
    res = run_bass_kernel_spmd(
        nc, make_in_maps(inputs), core_ids=list(range(B)), **kwargs
    )
    y = np.stack([res.results[b]["y"] for b in range(B)], axis=0)
    return y.reshape(B, H, W, C).astype(np.float32), res


def kernel(**inputs) -> np.ndarray:
    y, _ = run(inputs)
    return y
